# revision 31
# baseline (speedup 1.0000x reference)
"""Bahdanau attention kernel for Trainium2, SPMD across 8 NeuronCores.

Full inputs in, full outputs out. Sharding: data-parallel over batch B=64
(8 batches per core); W1/W2/V replicated.

Per-core pipeline (BL=8 batches, HW=1024 rows/batch, C=A=512), all bf16
on the TensorEngine, f32 accumulation:
  per batch-chunk k:
    DMA features chunk -> cast bf16 -> one X-bar DMA transpose -> 32
    W1-stationary matmuls -> tanh(+h_projT bias) on ACT -> V-dot matmuls
    (V replicated into 8 columns so scores land [8, 512] rows-equal)
    -> exp(+bV) -> mask-mult -> row-sum -> alpha row, and the batch's
    context matmuls (selector-column weights) overlap the next chunk.
  Softmax skips max-subtraction (scores are O(1); exact same math).
"""

import numpy as np
import ml_dtypes

import concourse.bass as bass
import concourse.mybir as mybir
import concourse.tile as tile
from concourse import bacc
from concourse.bass import ts, ds
from concourse.bass_utils import run_bass_kernel_spmd

F32 = mybir.dt.float32
BF16 = mybir.dt.bfloat16

B, H, W, C = 64, 32, 32, 512
A = 512
HID = 512
NCORES = 8
BL = B // NCORES          # batches per core
RPB = H * W               # rows (spatial positions) per batch
R = BL * RPB              # rows per core
P = 128
KT = C // P               # contraction tiles over C
AT = A // P               # tiles over attention dim
SUB = RPB // P            # 128-row subtiles per batch
HALF = 512                # psum free-dim half (PSUM bank = 512 f32)

Tanh = mybir.ActivationFunctionType.Tanh
Exp = mybir.ActivationFunctionType.Exp


def build():
    nc = bacc.Bacc(None, target_bir_lowering=False)

    feat = nc.declare_dram_parameter("features", [R, C], F32, isOutput=False)
    hid = nc.declare_dram_parameter("hidden", [BL, HID], F32, isOutput=False)
    maskp = nc.declare_dram_parameter("mask", [BL, BL, RPB], BF16, isOutput=False)
    w1p = nc.declare_dram_parameter("W1", [C, A], F32, isOutput=False)
    w2p = nc.declare_dram_parameter("W2", [HID, A], F32, isOutput=False)
    b2p = nc.declare_dram_parameter("b2", [A], F32, isOutput=False)
    vp = nc.declare_dram_parameter("V", [A, 1], F32, isOutput=False)
    bvp = nc.declare_dram_parameter("bV", [BL, 1], F32, isOutput=False)
    idp = nc.declare_dram_parameter("ident8", [BL, BL], BF16, isOutput=False)
    octx = nc.declare_dram_parameter("out_ctx", [BL, C], F32, isOutput=True)
    oalpha = nc.declare_dram_parameter("out_alpha", [BL, RPB], F32, isOutput=True)

    with tile.TileContext(nc) as tc:
        with (
            tc.tile_pool(name="const", bufs=1) as cp,
            tc.tile_pool(name="wstage", bufs=1) as wsp,
            tc.tile_pool(name="xnat", bufs=1) as xp,
            tc.tile_pool(name="stream", bufs=2) as sp,
            tc.tile_pool(name="pf", bufs=2, space=bass.MemorySpace.PSUM) as pfp,
            tc.tile_pool(name="psc", bufs=1, space=bass.MemorySpace.PSUM) as pscp,
            tc.tile_pool(name="pbdt", bufs=2, space=bass.MemorySpace.PSUM) as pbp,
            tc.tile_pool(name="pp", bufs=1, space=bass.MemorySpace.PSUM) as ppp,
        ):
            # ---------------- constants / small setup ---------------------
            w1f = wsp.tile([P, KT, A], F32, tag="wstage")
            nc.gpsimd.dma_start(w1f[:], w1p[:].rearrange("(kt p) a -> p kt a", p=P))
            w1b = cp.tile([P, KT, A], BF16)
            nc.vector.tensor_copy(w1b[:], w1f[:])

            w2f = wsp.tile([P, KT, A], F32, tag="wstage")
            nc.gpsimd.dma_start(w2f[:], w2p[:].rearrange("(kt p) a -> p kt a", p=P))
            w2b = cp.tile([P, KT, A], BF16)
            nc.vector.tensor_copy(w2b[:], w2f[:])

            vf = cp.tile([P, AT, 1], F32)
            nc.gpsimd.dma_start(vf[:], vp[:].rearrange("(kt p) one -> p kt one", p=P))
            vb = cp.tile([P, AT, 1], BF16)
            nc.vector.tensor_copy(vb[:], vf[:])
            # V replicated into all 8 stationary columns
            vrep = cp.tile([P, AT, BL], BF16)
            for a in range(AT):
                for m in range(BL):
                    nc.vector.tensor_copy(vrep[:, a, m : m + 1], vb[:, a, :])

            # b2 (+b1, folded host-side) as a K=1 matmul row
            b2row = cp.tile([1, A], F32)
            nc.gpsimd.dma_start(b2row[:], b2p[:].rearrange("(one a) -> one a", one=1))
            b2rb = cp.tile([1, A], BF16)
            nc.vector.tensor_copy(b2rb[:], b2row[:])
            ones8 = cp.tile([1, BL], BF16)
            nc.vector.memset(ones8[:], 1.0)

            bvb = cp.tile([BL, 1], F32)
            nc.gpsimd.dma_start(bvb[:], bvp[:])

            hidf = cp.tile([BL, HID], F32)
            nc.gpsimd.dma_start(hidf[:], hid[:])
            hidb = cp.tile([BL, HID], BF16)
            nc.vector.tensor_copy(hidb[:], hidf[:])

            # mask, host-replicated across 8 partitions, bf16 (0/1 exact)
            maskr = cp.tile([BL, BL, RPB], BF16)
            nc.gpsimd.dma_start(maskr[:], maskp[:])

            ident = cp.tile([BL, BL], BF16)
            nc.gpsimd.dma_start(ident[:], idp[:])

            # hiddenT: [HID, BL] as [P, KT, BL] via PE transpose
            p_hT = ppp.tile([P, KT, BL], BF16, tag="scratch")
            for kt in range(KT):
                nc.tensor.transpose(p_hT[:, kt, :], hidb[0:BL, ts(kt, P)], ident[:])
            hidTb = cp.tile([P, KT, BL], BF16)
            nc.vector.tensor_copy(hidTb[:], p_hT[:])

            # h_projT[a_part, a_tile, b] = (hidden @ W2 + b2 + b1)^T
            p_h = ppp.tile([P, AT, BL], F32, tag="scratch")
            for a in range(AT):
                for kk in range(KT):
                    nc.tensor.matmul(
                        p_h[:, a, :],
                        w2b[:, kk, ts(a, P)],
                        hidTb[:, kk, :],
                        start=(kk == 0),
                        stop=False,
                    )
                nc.tensor.matmul(
                    p_h[:, a, :],
                    b2rb[0:1, ts(a, P)],
                    ones8[:],
                    start=False,
                    stop=True,
                )
            hprojT = cp.tile([P, AT, BL], F32)
            nc.vector.tensor_copy(hprojT[:], p_h[:])


            # selector-column weights for the context matmul
            bd = cp.tile([P, BL * SUB, BL], BF16)
            nc.vector.memset(bd[:], 0.0)

            p_ctx = ppp.tile([BL, C], F32, tag="pctx")

            # ---------------- main loop over batches (1024-row chunks) -----
            # Per-chunk feature tiles (separate tiles -> no false WAR between
            # the next chunk's load and this chunk's context matmuls).
            # SWDGE casts f32->bf16 inline during the load. Transposes are
            # issued two chunks ahead so their completion latency hides.
            xcs = []
            for k in range(BL):
                t = xp.tile([P, SUB, C], BF16, tag=f"x16_{k}")
                xcs.append(t)
            xTs = []

            # all feature loads up front in half-chunk pieces: the SWDGE
            # queue stays pure, so loads stream back-to-back at HBM rate
            HROWS = RPB // 2
            for k in range(BL):
                for h in range(2):
                    nc.gpsimd.dma_start(
                        xcs[k][:, ds(h * 4, 4), :],
                        feat[
                            k * RPB + h * HROWS : k * RPB + (h + 1) * HROWS, :
                        ].rearrange("(s p) c -> p s c", p=P),
                    )
            # all transposes up front on sync, in halves; xT slot WARs pace
            # them to run 2-3 chunks ahead of the compute automatically
            for k in range(BL):
                xT = sp.tile([P, SUB, KT, P], BF16, tag="xT", bufs=3)
                for h in range(2):
                    nc.sync.dma_start(
                        xT[:, ds(h * 4, 4), :, :],
                        xcs[k][:, ds(h * 4, 4), :],
                        transpose=True,
                    )
                xTs.append(xT)

            for k in range(BL):
                xT = xTs[k]
                tanh_t = sp.tile([P, AT, RPB], BF16, tag="tanh")
                for a in range(AT):
                    for half in range(2):
                        pf = pfp.tile([P, HALF], F32, tag="pf")
                        for kc in range(KT):
                            nc.tensor.matmul(
                                pf[:],
                                w1b[:, kc, ts(a, P)],
                                xT[:, ds(half * 4, 4), kc, :],
                                start=(kc == 0),
                                stop=(kc == KT - 1),
                            )
                        nc.scalar.activation(
                            tanh_t[:, a, ts(half, HALF)],
                            pf[:],
                            Tanh,
                            bias=hprojT[:, a, k : k + 1],
                        )

                # V-dot; V replicated -> psum rows all equal the score row
                psc0 = pscp.tile([BL, HALF], F32, tag="psc0")
                psc1 = pscp.tile([BL, HALF], F32, tag="psc1")
                for half, psc in enumerate((psc0, psc1)):
                    for a in range(AT):
                        nc.tensor.matmul(
                            psc[:],
                            vrep[:, a, :],
                            tanh_t[:, a, ts(half, HALF)],
                            start=(a == 0),
                            stop=(a == AT - 1),
                        )

                # exp(+bV), mask, row-sum, normalize; rows all equal so
                # everything runs at partition base 0 and 1/S folds into
                # the context weights (context needs no epilogue divide)
                esc = sp.tile([BL, RPB], F32, tag="esc")
                nc.scalar.activation(esc[:, ts(0, HALF)], psc0[:], Exp, bias=bvb[:])
                nc.scalar.activation(esc[:, ts(1, HALF)], psc1[:], Exp, bias=bvb[:])
                nc.vector.tensor_tensor(
                    esc[:], esc[:], maskr[:, k, :], mybir.AluOpType.mult
                )
                rs8 = sp.tile([BL, 1], F32, tag="rs8")
                nc.vector.tensor_reduce(
                    rs8[:], esc[:], mybir.AxisListType.X, mybir.AluOpType.add
                )
                ri8 = sp.tile([BL, 1], F32, tag="ri8")
                nc.vector.reciprocal(ri8[:], rs8[:])
                nc.vector.tensor_scalar_mul(esc[:], esc[:], ri8[:])
                nc.gpsimd.dma_start(oalpha[k : k + 1, :], esc[0:1, :])

                # context weights: transpose alpha row into bd column k
                wmb = sp.tile([BL, RPB], BF16, tag="wmb")
                nc.vector.tensor_copy(wmb[:], esc[:])
                p_bdT = pbp.tile([P, SUB, BL], BF16, tag="pbdT")
                for s in range(SUB):
                    nc.tensor.transpose(
                        p_bdT[:, s, :], wmb[0:BL, ts(s, P)], ident[:]
                    )
                nc.vector.tensor_copy(
                    bd[:, ds(k * SUB, SUB), k : k + 1], p_bdT[:, :, k : k + 1]
                )

                # context matmuls for this batch (one group across all k)
                for s in range(SUB):
                    nc.tensor.matmul(
                        p_ctx[:],
                        bd[:, k * SUB + s, :],
                        xcs[k][:, s, :],
                        start=(k == 0 and s == 0),
                        stop=(k == BL - 1 and s == SUB - 1),
                    )

            # ---------------- outputs -------------------------------------
            ctxf = cp.tile([BL, C], F32)
            nc.vector.tensor_copy(ctxf[:], p_ctx[:])
            nc.sync.dma_start(octx[:], ctxf[:])

    nc.compile()
    return nc


_NC = None


def _get_nc():
    global _NC
    if _NC is None:
        _NC = build()
    return _NC


def make_in_maps(inputs):
    feats = np.ascontiguousarray(inputs["features"], dtype=np.float32)
    hidden = np.ascontiguousarray(inputs["hidden"], dtype=np.float32)
    mask = np.ascontiguousarray(inputs["mask"]).astype(np.float32)
    w1 = np.ascontiguousarray(inputs["W1"], dtype=np.float32)
    b1 = np.ascontiguousarray(inputs["b1"], dtype=np.float32)
    w2 = np.ascontiguousarray(inputs["W2"], dtype=np.float32)
    b2 = np.ascontiguousarray(inputs["b2"], dtype=np.float32)
    v = np.ascontiguousarray(inputs["V"], dtype=np.float32)
    bv = np.ascontiguousarray(inputs["bV"], dtype=np.float32)

    in_maps = []
    for i in range(NCORES):
        sl = slice(i * BL, (i + 1) * BL)
        m = mask[sl].reshape(BL, RPB).astype(ml_dtypes.bfloat16)
        m_rep = np.ascontiguousarray(
            np.broadcast_to(m[None, :, :], (BL, BL, RPB))
        )
        in_maps.append(
            {
                "features": feats[sl].reshape(R, C),
                "hidden": hidden[sl],
                "mask": m_rep,
                "W1": w1,
                "W2": w2,
                "b2": b2 + b1,
                "V": v,
                "bV": np.full((BL, 1), float(bv.reshape(-1)[0]), dtype=np.float32),
                "ident8": np.eye(BL, dtype=ml_dtypes.bfloat16),
            }
        )
    return in_maps


def run(inputs, trace=False, trace_kwargs=None):
    in_maps = make_in_maps(inputs)
    nc = _get_nc()
    kw = {}
    if trace:
        kw["trace"] = True
        if trace_kwargs:
            kw["trace_kwargs"] = trace_kwargs
    res = run_bass_kernel_spmd(nc, in_maps, list(range(NCORES)), **kw)

    ctx = np.concatenate([r["out_ctx"] for r in res.results], axis=0)
    alpha = np.concatenate([r["out_alpha"] for r in res.results], axis=0)
    alpha = alpha.reshape(B, H, W)
    return (ctx.astype(np.float32), alpha.astype(np.float32)), res


def kernel(**inputs):
    (ctx, alpha), _ = run(inputs, trace=False)
    return ctx, alpha


# revision 35
# speedup vs baseline: 1.3159x; 1.3159x over previous
"""Bahdanau attention kernel for Trainium2, SPMD across 8 NeuronCores.

Full inputs in, full outputs out. Sharding: data-parallel over batch B=64
(8 batches per core); W1/W2/V replicated.

Per-core pipeline (BL=8 batches, HW=1024 rows/batch, C=A=512), bf16 on
the TensorEngine, f32 accumulation. Features are cast f32->bf16 inline
by the SWDGE load. The [rows, C] -> [C, rows] transpose runs on the
TensorEngine (identity matmuls) staged through PSUM, so no DMA sits on
the chunk critical path. Scores use V replicated into 8 stationary
columns (psum rows all equal); softmax skips max-subtraction (scores are
O(1); identical math), and 1/sum folds into the context weights.
"""

import numpy as np
import ml_dtypes

import concourse.bass as bass
import concourse.mybir as mybir
import concourse.tile as tile
from concourse import bacc
from concourse.bass import ts, ds
from concourse.bass_utils import run_bass_kernel_spmd

F32 = mybir.dt.float32
BF16 = mybir.dt.bfloat16

B, H, W, C = 64, 32, 32, 512
A = 512
HID = 512
NCORES = 8
BL = B // NCORES          # batches per core
RPB = H * W               # rows (spatial positions) per batch
R = BL * RPB              # rows per core
P = 128
KT = C // P               # contraction tiles over C
AT = A // P               # tiles over attention dim
SUB = RPB // P            # 128-row subtiles per batch
HALF = 512                # psum free-dim half (PSUM bank = 512 f32)

Tanh = mybir.ActivationFunctionType.Tanh
Exp = mybir.ActivationFunctionType.Exp


def build():
    nc = bacc.Bacc(None, target_bir_lowering=False)

    feat = nc.declare_dram_parameter("features", [R, C], F32, isOutput=False)
    hid = nc.declare_dram_parameter("hidden", [BL, HID], F32, isOutput=False)
    maskp = nc.declare_dram_parameter("mask", [BL, BL, RPB], BF16, isOutput=False)
    w1p = nc.declare_dram_parameter("W1", [C, A], F32, isOutput=False)
    w2p = nc.declare_dram_parameter("W2", [HID, A], F32, isOutput=False)
    b2p = nc.declare_dram_parameter("b2", [A], F32, isOutput=False)
    vp = nc.declare_dram_parameter("V", [A, 1], F32, isOutput=False)
    bvp = nc.declare_dram_parameter("bV", [BL, 1], F32, isOutput=False)
    idp = nc.declare_dram_parameter("ident8", [BL, BL], BF16, isOutput=False)
    idp128 = nc.declare_dram_parameter("ident128", [P, P], BF16, isOutput=False)
    octx = nc.declare_dram_parameter("out_ctx", [BL, C], F32, isOutput=True)
    oalpha = nc.declare_dram_parameter("out_alpha", [BL, RPB], F32, isOutput=True)

    with tile.TileContext(nc) as tc:
        with (
            tc.tile_pool(name="const", bufs=1) as cp,
            tc.tile_pool(name="wstage", bufs=1) as wsp,
            tc.tile_pool(name="xnat", bufs=1) as xp,
            tc.tile_pool(name="stream", bufs=2) as sp,
            tc.tile_pool(name="pf", bufs=2, space=bass.MemorySpace.PSUM) as pfp,
            tc.tile_pool(name="psc", bufs=1, space=bass.MemorySpace.PSUM) as pscp,
            tc.tile_pool(name="pbdt", bufs=1, space=bass.MemorySpace.PSUM) as pbp,
            tc.tile_pool(name="pp", bufs=2, space=bass.MemorySpace.PSUM) as ppp,
        ):
            # ---------------- constants / small setup ---------------------
            vf = cp.tile([P, AT, 1], F32)
            nc.gpsimd.dma_start(vf[:], vp[:].rearrange("(kt p) one -> p kt one", p=P))
            vb = cp.tile([P, AT, 1], BF16)
            nc.vector.tensor_copy(vb[:], vf[:])
            vrep = cp.tile([P, AT, BL], BF16)
            for a in range(AT):
                for m in range(BL):
                    nc.vector.tensor_copy(vrep[:, a, m : m + 1], vb[:, a, :])

            w1f = wsp.tile([P, KT, A], F32, tag="wstage")
            nc.gpsimd.dma_start(w1f[:], w1p[:].rearrange("(kt p) a -> p kt a", p=P))
            w1b = cp.tile([P, KT, A], BF16)
            nc.vector.tensor_copy(w1b[:], w1f[:])

            w2f = wsp.tile([P, KT, A], F32, tag="wstage")
            nc.gpsimd.dma_start(w2f[:], w2p[:].rearrange("(kt p) a -> p kt a", p=P))
            w2b = cp.tile([P, KT, A], BF16)
            nc.vector.tensor_copy(w2b[:], w2f[:])

            b2row = cp.tile([1, A], F32)
            nc.gpsimd.dma_start(b2row[:], b2p[:].rearrange("(one a) -> one a", one=1))
            b2rb = cp.tile([1, A], BF16)
            nc.vector.tensor_copy(b2rb[:], b2row[:])
            ones8 = cp.tile([1, BL], BF16)
            nc.vector.memset(ones8[:], 1.0)

            bvb = cp.tile([BL, 1], F32)
            nc.gpsimd.dma_start(bvb[:], bvp[:])

            hidf = cp.tile([BL, HID], F32)
            nc.gpsimd.dma_start(hidf[:], hid[:])
            hidb = cp.tile([BL, HID], BF16)
            nc.vector.tensor_copy(hidb[:], hidf[:])

            maskr = cp.tile([BL, BL, RPB], BF16)
            nc.gpsimd.dma_start(maskr[:], maskp[:])

            ident = cp.tile([BL, BL], BF16)
            nc.gpsimd.dma_start(ident[:], idp[:])
            ident128 = cp.tile([P, P], BF16)
            nc.gpsimd.dma_start(ident128[:], idp128[:])

            # hiddenT: [HID, BL] as [P, KT, BL] via PE transpose
            p_hT = ppp.tile([P, KT, BL], BF16, tag="scratch")
            for kt in range(KT):
                nc.tensor.transpose(p_hT[:, kt, :], hidb[0:BL, ts(kt, P)], ident[:])
            hidTb = cp.tile([P, KT, BL], BF16)
            nc.vector.tensor_copy(hidTb[:], p_hT[:])

            # h_projT[a_part, a_tile, b] = (hidden @ W2 + b2 + b1)^T
            p_h = ppp.tile([P, AT, BL], F32, tag="scratch")
            for a in range(AT):
                for kk in range(KT):
                    nc.tensor.matmul(
                        p_h[:, a, :],
                        w2b[:, kk, ts(a, P)],
                        hidTb[:, kk, :],
                        start=(kk == 0),
                        stop=False,
                    )
                nc.tensor.matmul(
                    p_h[:, a, :],
                    b2rb[0:1, ts(a, P)],
                    ones8[:],
                    start=False,
                    stop=True,
                )
            hprojT = cp.tile([P, AT, BL], F32)
            nc.vector.tensor_copy(hprojT[:], p_h[:])

            # selector-column weights for the context matmul
            bd = cp.tile([P, BL * SUB, BL], BF16)
            nc.vector.memset(bd[:], 0.0)

            p_ctx = ppp.tile([BL, C], F32, tag="pctx", bufs=1)

            # per-chunk feature tiles; SWDGE casts f32->bf16 during the load
            xcs = []
            for k in range(BL):
                t = xp.tile([P, SUB, C], BF16, tag=f"x16_{k}")
                xcs.append(t)
            for k in range(BL):
                for h in range(2):
                    nc.gpsimd.dma_start(
                        xcs[k][:, ds(h * 4, 4), :],
                        feat[
                            k * RPB + h * (RPB // 2) : k * RPB
                            + (h + 1) * (RPB // 2),
                            :,
                        ].rearrange("(s p) c -> p s c", p=P),
                    )

            xTs = []

            def pe_transpose_chunk(kk):
                # xcs[kk] [row%128, s, c] -> xT [c%128, s, kt, row%128]
                # 8 consecutive (s, kt) [128,128] transposes fill one psum
                # bank in xT memory order; one DVE copy drains each bank.
                xT = sp.tile([P, SUB, KT, P], BF16, tag="xT", bufs=2)
                for g in range(4):
                    pt = ppp.tile([P, 2, KT, P], BF16, tag="scratch")
                    for s2 in range(2):
                        s = g * 2 + s2
                        for kt in range(KT):
                            nc.tensor.transpose(
                                pt[:, s2, kt, :],
                                xcs[kk][:, s, ts(kt, P)],
                                ident128[:],
                            )
                    nc.vector.tensor_copy(xT[:, ds(g * 2, 2), :, :], pt[:])
                xTs.append(xT)

            pe_transpose_chunk(0)

            # ---------------- main loop over batches (1024-row chunks) -----
            for k in range(BL):
                xT = xTs[k]
                tanh_t = sp.tile([P, AT, RPB], BF16, tag="tanh")
                for a in range(AT):
                    for half in range(2):
                        pf = pfp.tile([P, HALF], F32, tag="pf")
                        for kc in range(KT):
                            nc.tensor.matmul(
                                pf[:],
                                w1b[:, kc, ts(a, P)],
                                xT[:, ds(half * 4, 4), kc, :],
                                start=(kc == 0),
                                stop=(kc == KT - 1),
                            )
                        nc.scalar.activation(
                            tanh_t[:, a, ts(half, HALF)],
                            pf[:],
                            Tanh,
                            bias=hprojT[:, a, k : k + 1],
                        )

                # transposes for the next chunk ride between compute blocks
                if k + 1 <= BL - 1:
                    pe_transpose_chunk(k + 1)

                # V-dot; V replicated -> psum rows all equal the score row
                psc0 = pscp.tile([BL, HALF], F32, tag="psc0")
                psc1 = pscp.tile([BL, HALF], F32, tag="psc1")
                for half, psc in enumerate((psc0, psc1)):
                    for a in range(AT):
                        nc.tensor.matmul(
                            psc[:],
                            vrep[:, a, :],
                            tanh_t[:, a, ts(half, HALF)],
                            start=(a == 0),
                            stop=(a == AT - 1),
                        )

                # exp(+bV), mask, row-sum, normalize at partition base 0;
                # 1/S folds into the context weights
                esc = sp.tile([BL, RPB], F32, tag="esc")
                nc.scalar.activation(esc[:, ts(0, HALF)], psc0[:], Exp, bias=bvb[:])
                nc.scalar.activation(esc[:, ts(1, HALF)], psc1[:], Exp, bias=bvb[:])
                nc.vector.tensor_tensor(
                    esc[:], esc[:], maskr[:, k, :], mybir.AluOpType.mult
                )
                rs8 = sp.tile([BL, 1], F32, tag="rs8")
                nc.vector.tensor_reduce(
                    rs8[:], esc[:], mybir.AxisListType.X, mybir.AluOpType.add
                )
                ri8 = sp.tile([BL, 1], F32, tag="ri8")
                nc.vector.reciprocal(ri8[:], rs8[:])
                nc.vector.tensor_scalar_mul(esc[:], esc[:], ri8[:])
                nc.gpsimd.dma_start(oalpha[k : k + 1, :], esc[0:1, :])

                # context weights: transpose alpha row into bd column k
                wmb = sp.tile([BL, RPB], BF16, tag="wmb")
                nc.vector.tensor_copy(wmb[:], esc[:])
                p_bdT = pbp.tile([P, SUB, BL], BF16, tag="pbdT")
                for s in range(SUB):
                    nc.tensor.transpose(
                        p_bdT[:, s, :], wmb[0:BL, ts(s, P)], ident[:]
                    )
                nc.vector.tensor_copy(
                    bd[:, ds(k * SUB, SUB), k : k + 1], p_bdT[:, :, k : k + 1]
                )

                # context matmuls for this batch (one group across all k)
                for s in range(SUB):
                    nc.tensor.matmul(
                        p_ctx[:],
                        bd[:, k * SUB + s, :],
                        xcs[k][:, s, :],
                        start=(k == 0 and s == 0),
                        stop=(k == BL - 1 and s == SUB - 1),
                    )

            # ---------------- outputs -------------------------------------
            ctxf = cp.tile([BL, C], F32)
            nc.vector.tensor_copy(ctxf[:], p_ctx[:])
            nc.gpsimd.dma_start(octx[:], ctxf[:])

    nc.compile()
    return nc


_NC = None


def _get_nc():
    global _NC
    if _NC is None:
        _NC = build()
    return _NC


def make_in_maps(inputs):
    feats = np.ascontiguousarray(inputs["features"], dtype=np.float32)
    hidden = np.ascontiguousarray(inputs["hidden"], dtype=np.float32)
    mask = np.ascontiguousarray(inputs["mask"]).astype(np.float32)
    w1 = np.ascontiguousarray(inputs["W1"], dtype=np.float32)
    b1 = np.ascontiguousarray(inputs["b1"], dtype=np.float32)
    w2 = np.ascontiguousarray(inputs["W2"], dtype=np.float32)
    b2 = np.ascontiguousarray(inputs["b2"], dtype=np.float32)
    v = np.ascontiguousarray(inputs["V"], dtype=np.float32)
    bv = np.ascontiguousarray(inputs["bV"], dtype=np.float32)

    in_maps = []
    for i in range(NCORES):
        sl = slice(i * BL, (i + 1) * BL)
        m = mask[sl].reshape(BL, RPB).astype(ml_dtypes.bfloat16)
        m_rep = np.ascontiguousarray(
            np.broadcast_to(m[None, :, :], (BL, BL, RPB))
        )
        in_maps.append(
            {
                "features": feats[sl].reshape(R, C),
                "hidden": hidden[sl],
                "mask": m_rep,
                "W1": w1,
                "W2": w2,
                "b2": b2 + b1,
                "V": v,
                "bV": np.full((BL, 1), float(bv.reshape(-1)[0]), dtype=np.float32),
                "ident8": np.eye(BL, dtype=ml_dtypes.bfloat16),
                "ident128": np.eye(P, dtype=ml_dtypes.bfloat16),
            }
        )
    return in_maps


def run(inputs, trace=False, trace_kwargs=None):
    in_maps = make_in_maps(inputs)
    nc = _get_nc()
    kw = {}
    if trace:
        kw["trace"] = True
        if trace_kwargs:
            kw["trace_kwargs"] = trace_kwargs
    res = run_bass_kernel_spmd(nc, in_maps, list(range(NCORES)), **kw)

    ctx = np.concatenate([r["out_ctx"] for r in res.results], axis=0)
    alpha = np.concatenate([r["out_alpha"] for r in res.results], axis=0)
    alpha = alpha.reshape(B, H, W)
    return (ctx.astype(np.float32), alpha.astype(np.float32)), res


def kernel(**inputs):
    (ctx, alpha), _ = run(inputs, trace=False)
    return ctx, alpha
